# revision 1
# baseline (speedup 1.0000x reference)
"""Chamfer distance kernel for 8 Trainium2 NeuronCores (Bass/Tile).

Problem: xyz1, xyz2: (4, 8192, 3) fp32. Outputs dist1, dist2: (4, 8192) fp32,
the row-wise / column-wise minima of the pairwise squared-distance matrix
d[n,m] = max(||x_n||^2 + ||y_m||^2 - 2 x_n.y_m, 0), per batch.

Sharding: core c handles batch c//2 and half of the N rows (c%2). Each core
computes dist1 for its 4096 rows exactly, and a dist2 partial (min over its
4096 rows) for all 8192 columns; the host min-combines the two partials.

Per-core kernel: distance tiles are produced by ONE bf16 matmul each, using
K=24 augmented vectors: bf16x3 decompositions of x, of -2*y and of the two
squared norms, ordered so the large terms cancel early in the fp32 PSUM
accumulation (fp32-faithful, representation residual ~2^-27).

Both outputs are row-min reductions of some orientation of d, so the kernel
computes BOTH orientations (d = xa^T ya and d^T = ya^T xa — the TensorEngine
has large headroom) in [128, 4x512] PSUM groups (4 banks), and the DVE does a
single tensor_reduce(min) per group. dist1 comes from the A-orientation
reduces, dist2 from the B-orientation ones. Group partials land in columns of
a small SBUF tile; one strided reduce + relu finishes each output.
"""

from contextlib import ExitStack

import numpy as np
import ml_dtypes

B, N, M = 4, 8192, 8192
NCORES = 8
NLOC = N // 2          # rows of xyz1 per core
P = 128                # partitions
FD = 512               # matmul free dim (one PSUM bank of fp32)
GRP = 4                # m-chunks per PSUM group (4 banks = [128, 2048])
KAUG = 24

_BF16 = ml_dtypes.bfloat16


def _decomp3(v):
    """fp32/fp64 array -> three bf16 planes summing to v (residual ~2^-27)."""
    v = v.astype(np.float32)
    h = v.astype(_BF16)
    r = v - h.astype(np.float32)
    m = r.astype(_BF16)
    r2 = r - m.astype(np.float32)
    l = r2.astype(_BF16)
    return h, m, l


def _build_aug(x, y):
    """x: [Nl,3] fp32, y: [Mm,3] fp32 -> (xa [KAUG,Nl] bf16, ya [KAUG,Mm] bf16).

    d[n,m] = sum_k xa[k,n]*ya[k,m] up to bf16x3 residuals. Slot order puts the
    large mutually-cancelling terms first so fp32 PSUM accumulation stays
    accurate near d ~ 0.
    """
    nl, mm = x.shape[0], y.shape[0]
    nx = (x.astype(np.float64) ** 2).sum(axis=1)
    ny = (y.astype(np.float64) ** 2).sum(axis=1)
    xh, xm, xl = _decomp3(x)
    y2 = (-2.0 * y.astype(np.float64)).astype(np.float32)
    yh, ym, yl = _decomp3(y2)
    nxh, nxm, nxl = _decomp3(nx)
    nyh, nym, nyl = _decomp3(ny)

    one_n = np.ones(nl, dtype=_BF16)
    one_m = np.ones(mm, dtype=_BF16)

    xa = np.empty((KAUG, nl), dtype=_BF16)
    ya = np.empty((KAUG, mm), dtype=_BF16)
    k = 0

    def slot(xv, yv):
        nonlocal k
        xa[k] = xv
        ya[k] = yv
        k += 1

    slot(nxh, one_m)
    slot(one_n, nyh)
    for c in range(3):
        slot(xh[:, c], yh[:, c])
    slot(nxm, one_m)
    slot(one_n, nym)
    for c in range(3):
        slot(xh[:, c], ym[:, c])
    for c in range(3):
        slot(xm[:, c], yh[:, c])
    slot(nxl, one_m)
    slot(one_n, nyl)
    for c in range(3):
        slot(xh[:, c], yl[:, c])
    for c in range(3):
        slot(xm[:, c], ym[:, c])
    for c in range(3):
        slot(xl[:, c], yh[:, c])
    assert k == KAUG
    return xa, ya


def build_bass(
    nloc=NLOC, m_total=M, repeat=1, grp=None, psum_bufs=2, copy_mod=1000,
    spool_bufs=3, pair=1, s2_mode="f32", s_pad=16, s2_pad=0, phase=0,
):
    """Build + compile the per-core Bass program.

    repeat>1 wraps the main compute in a dynamic loop executing it `repeat`
    times — used only to measure per-iteration HW time above the PJRT
    dispatch noise floor.

    Group reduction runs down one of two paths (HW-measured: every DVE
    reduce-class op is capped at 1 elem/lane/cycle — accumulating ops never
    get the 2x perf modes — so the only win left is overlapping the
    otherwise-idle ScalarE):
      - copy path (groups with idx % copy_mod != copy_mod-1): ScalarE copies
        the PSUM group to SBUF (~2.0us), then the DVE reduces from SBUF with
        tensor_scalar(max 0, min-accum) (~2.16us vs ~2.30us from PSUM).
      - direct path (remaining groups): plain DVE tensor_reduce from PSUM.
    Default copy_mod=1000 routes every group through the copy path: measured
    ~558-569us/core vs ~586us all-direct (copy_mod=1) and ~573us at
    copy_mod=4; the DVE 1x streaming bound for the 256 groups is ~546us.

    s_pad pads the SBUF copy tiles by s_pad fp32 per chunk so consecutive
    pool slots land on a different SBUF bank phase — cuts ScalarE-write vs
    DVE-read bank conflicts (s_pad=16 beat s_pad=0 by 22-62us in paired
    same-process A/Bs; 32 was worse). Absolute timings drift ~+/-40us
    between sessions (device/terminal state), so configs were always
    compared within one process.
    """
    import concourse.bacc as bacc
    import concourse.tile as tile
    import concourse.mybir as mybir

    f32 = mybir.dt.float32
    bf16 = mybir.dt.bfloat16
    Alu = mybir.AluOpType
    X = mybir.AxisListType.X
    XY = mybir.AxisListType.XY
    XYZ = mybir.AxisListType.XYZ

    grp = GRP if grp is None else grp
    ntile_a = nloc // P          # weight tiles, orientation A (dist1 rows)
    ngrp_a = m_total // (grp * FD)   # reduce groups per A weight tile
    ntile_b = m_total // P       # weight tiles, orientation B (dist2 rows)
    ngrp_b = nloc // (grp * FD)      # reduce groups per B weight tile

    nc = bacc.Bacc("TRN2", target_bir_lowering=False, debug=False)
    xa_d = nc.dram_tensor("xa", [KAUG, nloc], bf16, kind="ExternalInput")
    ya_d = nc.dram_tensor("ya", [KAUG, m_total], bf16, kind="ExternalInput")
    d1_d = nc.dram_tensor("d1", [P, ntile_a], f32, kind="ExternalOutput")
    d2_d = nc.dram_tensor("d2", [P, ntile_b], f32, kind="ExternalOutput")

    with tile.TileContext(nc) as tc, ExitStack() as ctx:
        singles = ctx.enter_context(tc.tile_pool(name="singles", bufs=1))
        psum = ctx.enter_context(
            tc.tile_pool(name="psum", bufs=psum_bufs, space="PSUM")
        )

        # chunked loads so the first matmuls start before the full tensors land
        xa = singles.tile([KAUG, nloc], bf16)
        for i in range(4):
            sl = slice(i * nloc // 4, (i + 1) * nloc // 4)
            nc.sync.dma_start(out=xa[:, sl], in_=xa_d.ap()[:, sl])
        ya = singles.tile([KAUG, m_total], bf16)
        for i in range(4):
            sl = slice(i * m_total // 4, (i + 1) * m_total // 4)
            nc.sync.dma_start(out=ya[:, sl], in_=ya_d.ap()[:, sl])

        if phase:
            # dummy allocation shifting the SBUF base address (bank phase) of
            # everything allocated after it
            ph = singles.tile([P, phase], f32, name="ph")
            nc.vector.memset(ph, 0.0)

        spool = ctx.enter_context(tc.tile_pool(name="spool", bufs=spool_bufs))
        s2pool = ctx.enter_context(tc.tile_pool(name="s2pool", bufs=2))

        # pair>1 fuses consecutive PSUM groups into one wide SBUF tile and a
        # single DVE reduce op (measured slower: coarser deps stall the
        # pipeline; pair=1 is the shipped config).
        PAIR = pair
        c1 = singles.tile([P, ntile_a * ngrp_a], f32)
        c2 = singles.tile([P, ntile_b * ngrp_b], f32)
        d1t = singles.tile([P, ntile_a], f32)
        d2t = singles.tile([P, ntile_b], f32)

        nc.gpsimd.memset(c1, 3.0e38)
        nc.gpsimd.memset(c2, 3.0e38)

        gidx = [0]  # global pair counter for path assignment

        def orientation(wt_count, grp_count, w_sb, mv_sb, cols):
            """One orientation: wt_count weight tiles x grp_count reduce
            groups; rows of the output come from w_sb, the reduction runs
            over all of mv_sb."""
            assert grp_count % PAIR == 0

            for it in range(wt_count):

                def emit_mms(g):
                    pt = psum.tile([P, grp, FD], f32, name="pt", tag="pt")
                    for j in range(grp):
                        nc.tensor.matmul(
                            pt[:, j, :],
                            w_sb[:, it * P : (it + 1) * P],
                            mv_sb[:, (g * grp + j) * FD : (g * grp + j + 1) * FD],
                            start=True,
                            stop=True,
                        )
                    return pt

                for gp in range(grp_count // PAIR):
                    use_copy = gidx[0] % copy_mod != copy_mod - 1
                    gidx[0] += 1
                    if use_copy:
                        # pair's min lands in the pair's first column slot;
                        # the second slot keeps its +BIG initialization.
                        col = it * grp_count + gp * PAIR
                        if s_pad:
                            s_full = spool.tile(
                                [P, PAIR, grp, FD + s_pad], f32,
                                name="s_full", tag="s",
                            )
                            s = s_full[:, :, :, :FD]
                        else:
                            s = spool.tile(
                                [P, PAIR, grp, FD], f32, name="s", tag="s"
                            )
                        for q in range(PAIR):
                            pt = emit_mms(gp * PAIR + q)
                            nc.scalar.copy(out=s[:, q], in_=pt)
                        if s2_mode == "reduce":
                            nc.vector.tensor_reduce(
                                out=cols[:, col : col + 1],
                                in_=s,
                                axis=XYZ if PAIR > 1 else XY,
                                op=Alu.min,
                            )
                        else:
                            s2_dt = bf16 if s2_mode == "bf16" else f32
                            if s2_pad:
                                s2_full = s2pool.tile(
                                    [P, PAIR, grp, FD + s2_pad], s2_dt,
                                    name="s2_full", tag="s2",
                                )
                                s2 = s2_full[:, :, :, :FD]
                            else:
                                s2 = s2pool.tile(
                                    [P, PAIR, grp, FD], s2_dt,
                                    name="s2", tag="s2",
                                )
                            nc.vector.tensor_scalar(
                                out=s2,
                                in0=s,
                                scalar1=0.0,
                                scalar2=None,
                                op0=Alu.max,
                                op1=Alu.min,
                                accum_out=cols[:, col : col + 1],
                            )
                    else:
                        for q in range(PAIR):
                            g = gp * PAIR + q
                            pt = emit_mms(g)
                            col = it * grp_count + g
                            nc.vector.tensor_reduce(
                                out=cols[:, col : col + 1],
                                in_=pt,
                                axis=XY,
                                op=Alu.min,
                            )

        def main_compute():
            orientation(ntile_a, ngrp_a, xa, ya, c1)
            orientation(ntile_b, ngrp_b, ya, xa, c2)

        if repeat == 1:
            main_compute()
        else:
            with tc.For_i(0, repeat, 1):
                main_compute()

        # finals: strided min over each weight-tile's group partials + relu
        nc.vector.tensor_reduce(
            out=d1t,
            in_=c1.rearrange("p (t g) -> p t g", g=ngrp_a),
            axis=X,
            op=Alu.min,
        )
        nc.vector.tensor_reduce(
            out=d2t,
            in_=c2.rearrange("p (t g) -> p t g", g=ngrp_b),
            axis=X,
            op=Alu.min,
        )
        nc.vector.tensor_scalar_max(out=d1t, in0=d1t, scalar1=0.0)
        nc.vector.tensor_scalar_max(out=d2t, in0=d2t, scalar1=0.0)

        nc.sync.dma_start(out=d1_d.ap(), in_=d1t)
        nc.sync.dma_start(out=d2_d.ap(), in_=d2t)

    nc.compile()
    return nc


_CACHED_NC = None


def _get_nc():
    global _CACHED_NC
    if _CACHED_NC is None:
        _CACHED_NC = build_bass()
    return _CACHED_NC


def _make_in_maps(xyz1, xyz2):
    xyz1 = np.asarray(xyz1, dtype=np.float32)
    xyz2 = np.asarray(xyz2, dtype=np.float32)
    in_maps = []
    for c in range(NCORES):
        b, h = divmod(c, 2)
        x = xyz1[b, h * NLOC : (h + 1) * NLOC]
        y = xyz2[b]
        xa, ya = _build_aug(x, y)
        in_maps.append({"xa": xa, "ya": ya})
    return in_maps


def _unshard(results):
    dist1 = np.empty((B, N), np.float32)
    dist2 = np.empty((B, M), np.float32)
    for c in range(NCORES):
        b, h = divmod(c, 2)
        dist1[b, h * NLOC : (h + 1) * NLOC] = np.asarray(results[c]["d1"]).T.ravel()
        d2p = np.asarray(results[c]["d2"]).T.ravel()
        if h == 0:
            dist2[b] = d2p
        else:
            np.minimum(dist2[b], d2p, out=dist2[b])
    return dist1, dist2


def kernel(xyz1, xyz2):
    from concourse.bass_utils import run_bass_kernel_spmd

    nc = _get_nc()
    in_maps = _make_in_maps(xyz1, xyz2)
    res = run_bass_kernel_spmd(nc, in_maps, core_ids=list(range(NCORES)))
    return _unshard(res.results)



# revision 2
# speedup vs baseline: 1.9456x; 1.9456x over previous
"""Chamfer distance kernel for 8 Trainium2 NeuronCores (Bass/Tile), v2.

Problem: xyz1, xyz2: (4, 8192, 3) fp32. Outputs dist1, dist2: (4, 8192) fp32,
the row-wise / column-wise minima of the pairwise squared-distance matrix
d[n,m] = max(||x_n||^2 + ||y_m||^2 - 2 x_n.y_m, 0), per batch.

Sharding: core c handles batch c//2 and half of the N rows (c%2). Each core
computes dist1 for its 4096 rows exactly, and a dist2 partial (min over its
4096 rows) for all 8192 columns; the host min-combines the two partials.

v2 design (vs the v1 both-orientations kernel at ~554us): the distance matrix
is computed ONCE per core (TensorE, K=24 bf16-augmented matmuls, fp32 PSUM),
ScalarE evacuates each PSUM group as relu'd BF16 into SBUF (1x, ~2.05us per
[128,2048]), and ALL min work runs on the DVE as bf16 SBUF tensor_tensor ops
which hit the 2x_1P perf mode (measured 2194ns for a [128,4096] TT min =
(58+2048)cyc @0.96GHz; reduce-class ops are stuck at 1x):

  - dist2: running elementwise-min accumulator colacc[128, 8192] over the 32
    row-tiles (2 TT ops per row-tile). The final cross-partition min runs as
    64 TensorE transposes of colacc blocks into bf16 bitcast views of the
    PSUM ring + 8 DVE tensor_reduce ops.
  - dist1: per row-tile a 2-level TT tree (8192 -> 2048) whose output lands
    in a [128, 8, 2048] batch buffer; every 8 row-tiles, 4 more batched tree
    levels (3D APs) + one small 1x reduce produce 8 row-min columns at once.

PSUM: one [128, 8, 512] f32 tile used as a ring; 4 matmuls (FD=512) fill a
[128, 4, 512] half, ScalarE reads it as one 2048-wide ACTIVATE. Subtile deps
give WAR/RAW ordering. The d2 transposes at the tail of the body overlap the
next repeat iteration's head, so the steady-state cost is engine-bound.
"""

from contextlib import ExitStack

import numpy as np
import ml_dtypes

B, N, M = 4, 8192, 8192
NCORES = 8
NLOC = N // 2          # rows of xyz1 per core
P = 128                # partitions
FD = 512               # matmul free dim (one PSUM bank of fp32)
KAUG = 24

_BF16 = ml_dtypes.bfloat16


def _decomp3(v):
    """fp32/fp64 array -> three bf16 planes summing to v (residual ~2^-27)."""
    v = v.astype(np.float32)
    h = v.astype(_BF16)
    r = v - h.astype(np.float32)
    m = r.astype(_BF16)
    r2 = r - m.astype(np.float32)
    l = r2.astype(_BF16)
    return h, m, l


def _build_aug(x, y):
    """x: [Nl,3] fp32, y: [Mm,3] fp32 -> (xa [KAUG,Nl] bf16, ya [KAUG,Mm] bf16).

    d[n,m] = sum_k xa[k,n]*ya[k,m] up to bf16x3 residuals. Slot order puts the
    large mutually-cancelling terms first so fp32 PSUM accumulation stays
    accurate near d ~ 0.
    """
    nl, mm = x.shape[0], y.shape[0]
    nx = (x.astype(np.float64) ** 2).sum(axis=1)
    ny = (y.astype(np.float64) ** 2).sum(axis=1)
    xh, xm, xl = _decomp3(x)
    y2 = (-2.0 * y.astype(np.float64)).astype(np.float32)
    yh, ym, yl = _decomp3(y2)
    nxh, nxm, nxl = _decomp3(nx)
    nyh, nym, nyl = _decomp3(ny)

    one_n = np.ones(nl, dtype=_BF16)
    one_m = np.ones(mm, dtype=_BF16)

    xa = np.empty((KAUG, nl), dtype=_BF16)
    ya = np.empty((KAUG, mm), dtype=_BF16)
    k = 0

    def slot(xv, yv):
        nonlocal k
        xa[k] = xv
        ya[k] = yv
        k += 1

    slot(nxh, one_m)
    slot(one_n, nyh)
    for c in range(3):
        slot(xh[:, c], yh[:, c])
    slot(nxm, one_m)
    slot(one_n, nym)
    for c in range(3):
        slot(xh[:, c], ym[:, c])
    for c in range(3):
        slot(xm[:, c], yh[:, c])
    slot(nxl, one_m)
    slot(one_n, nyl)
    for c in range(3):
        slot(xh[:, c], yl[:, c])
    for c in range(3):
        slot(xm[:, c], ym[:, c])
    for c in range(3):
        slot(xl[:, c], yh[:, c])
    assert k == KAUG
    return xa, ya


def build_bass(nloc=NLOC, m_total=M, repeat=1, s_bufs=3, s_pad=128, t_pad=64):
    """Build + compile the per-core Bass program.

    repeat>1 wraps the main compute in a dynamic loop executing it `repeat`
    times — used only to measure per-iteration HW time above the PJRT
    dispatch noise floor. The loop body is self-contained (accumulators are
    re-initialized inside), so outputs are repeat-invariant.

    s_pad/t_pad pad the bf16 SBUF tiles (elements per buf) to shift SBUF
    bank phase between pool slots (ScalarE-write vs DVE-read conflicts).
    """
    import concourse.bacc as bacc
    import concourse.tile as tile
    import concourse.mybir as mybir

    f32 = mybir.dt.float32
    bf16 = mybir.dt.bfloat16
    Alu = mybir.AluOpType
    Act = mybir.ActivationFunctionType
    X = mybir.AxisListType.X

    ntile = nloc // P             # 32 row-tiles
    ngrp = m_total // (4 * FD)    # 4 ScalarE groups of 2048 per row-tile
    nbatch = 8                    # row-tiles per dist1 batch phase
    nph = ntile // nbatch         # 4 batch phases
    nblk = m_total // P           # 64 transpose blocks for dist2

    nc = bacc.Bacc("TRN2", target_bir_lowering=False, debug=False)
    xa_d = nc.dram_tensor("xa", [KAUG, nloc], bf16, kind="ExternalInput")
    ya_d = nc.dram_tensor("ya", [KAUG, m_total], bf16, kind="ExternalInput")
    eye_d = nc.dram_tensor("eye", [P, P], bf16, kind="ExternalInput")
    d1_d = nc.dram_tensor("d1", [P, ntile], f32, kind="ExternalOutput")
    d2_d = nc.dram_tensor("d2", [P, nblk], f32, kind="ExternalOutput")

    with tile.TileContext(nc) as tc, ExitStack() as ctx:
        singles = ctx.enter_context(tc.tile_pool(name="singles", bufs=1))
        psum = ctx.enter_context(tc.tile_pool(name="psum", bufs=1, space="PSUM"))

        # chunked loads so the first matmuls start before the full tensors land
        xa = singles.tile([KAUG, nloc], bf16)
        for i in range(4):
            sl = slice(i * nloc // 4, (i + 1) * nloc // 4)
            nc.sync.dma_start(out=xa[:, sl], in_=xa_d.ap()[:, sl])
        ya = singles.tile([KAUG, m_total], bf16)
        for i in range(4):
            sl = slice(i * m_total // 4, (i + 1) * m_total // 4)
            nc.sync.dma_start(out=ya[:, sl], in_=ya_d.ap()[:, sl])
        eye = singles.tile([P, P], bf16)
        nc.sync.dma_start(out=eye, in_=eye_d.ap())

        spool = ctx.enter_context(tc.tile_pool(name="spool", bufs=s_bufs))
        tpool = ctx.enter_context(tc.tile_pool(name="tpool", bufs=2))

        colacc = singles.tile([P, m_total], bf16)
        rp = singles.tile([P, nbatch, 2048], bf16)
        c1 = singles.tile([P, ntile], f32)
        c2 = singles.tile([P, nblk], f32)
        d1t = singles.tile([P, ntile], f32)
        d2t = singles.tile([P, nblk], f32)

        # PSUM ring: [128, 8, 512] f32 = all 8 banks. Halves (4 slices =
        # 2048 f32) alternate between PE writes and one ScalarE ACTIVATE.
        pring = psum.tile([P, 8, FD], f32, name="pring")
        pring_bf = pring.bitcast(bf16)  # [128, 8, 1024] for d2 transposes

        def body():
            nc.gpsimd.memset(colacc, 3.0e38)

            for it in range(ntile):
                s = spool.tile(
                    [P, m_total], bf16, name="s", tag="s",
                    padded_shape=[P, m_total + s_pad],
                )
                for g in range(ngrp):
                    h = g % 2
                    for j in range(4):
                        nc.tensor.matmul(
                            pring[:, 4 * h + j, :],
                            xa[:, it * P : (it + 1) * P],
                            ya[:, (g * 4 + j) * FD : (g * 4 + j + 1) * FD],
                            start=True,
                            stop=True,
                        )
                    nc.scalar.activation(
                        out=s[:, g * 2048 : (g + 1) * 2048],
                        in_=pring[:, 4 * h : 4 * h + 4, :].rearrange(
                            "p a b -> p (a b)"
                        ),
                        func=Act.Relu,
                    )
                # dist2 column accumulation: 2x 4096-wide TT min
                for half in range(2):
                    sl = slice(half * 4096, (half + 1) * 4096)
                    nc.vector.tensor_tensor(
                        out=colacc[:, sl], in0=colacc[:, sl], in1=s[:, sl],
                        op=Alu.min,
                    )
                # dist1 per-tile tree: 8192 -> 4096 -> 2048 (into batch buf)
                t1 = tpool.tile(
                    [P, 4096], bf16, name="t1", tag="t1",
                    padded_shape=[P, 4096 + t_pad],
                )
                nc.vector.tensor_tensor(
                    out=t1, in0=s[:, :4096], in1=s[:, 4096:], op=Alu.min
                )
                nc.vector.tensor_tensor(
                    out=rp[:, it % nbatch, :],
                    in0=t1[:, :2048],
                    in1=t1[:, 2048:],
                    op=Alu.min,
                )
                # batched deep levels every nbatch row-tiles
                if it % nbatch == nbatch - 1:
                    ph = it // nbatch
                    w = 2048
                    while w > 128:
                        nc.vector.tensor_tensor(
                            out=rp[:, :, : w // 2],
                            in0=rp[:, :, : w // 2],
                            in1=rp[:, :, w // 2 : w],
                            op=Alu.min,
                        )
                        w //= 2
                    nc.vector.tensor_reduce(
                        out=c1[:, ph * nbatch : (ph + 1) * nbatch],
                        in_=rp[:, :, :128],
                        axis=X,
                        op=Alu.min,
                    )

            # dist2 finals: transpose colacc blocks into PSUM (bf16 views of
            # the ring slices), reduce each slice's 8 blocks on the DVE.
            for sl8 in range(8):
                for k in range(8):
                    blk = sl8 * 8 + k
                    nc.tensor.transpose(
                        out=pring_bf[:, sl8, k * P : (k + 1) * P],
                        in_=colacc[:, blk * P : (blk + 1) * P],
                        identity=eye,
                    )
                nc.vector.tensor_reduce(
                    out=c2[:, sl8 * 8 : (sl8 + 1) * 8],
                    in_=pring_bf[:, sl8, :].rearrange("p (b f) -> p b f", f=P),
                    axis=X,
                    op=Alu.min,
                )

            nc.vector.tensor_scalar_max(out=d1t, in0=c1, scalar1=0.0)
            nc.vector.tensor_scalar_max(out=d2t, in0=c2, scalar1=0.0)

        if repeat == 1:
            body()
        else:
            with tc.For_i(0, repeat, 1):
                body()

        nc.sync.dma_start(out=d1_d.ap(), in_=d1t)
        nc.sync.dma_start(out=d2_d.ap(), in_=d2t)

    nc.compile()
    return nc


_CACHED_NC = None


def _get_nc():
    global _CACHED_NC
    if _CACHED_NC is None:
        _CACHED_NC = build_bass()
    return _CACHED_NC


_EYE = np.eye(P, dtype=_BF16)


def _make_in_maps(xyz1, xyz2):
    xyz1 = np.asarray(xyz1, dtype=np.float32)
    xyz2 = np.asarray(xyz2, dtype=np.float32)
    in_maps = []
    for c in range(NCORES):
        b, h = divmod(c, 2)
        x = xyz1[b, h * NLOC : (h + 1) * NLOC]
        y = xyz2[b]
        xa, ya = _build_aug(x, y)
        in_maps.append({"xa": xa, "ya": ya, "eye": _EYE})
    return in_maps


def _unshard(results):
    dist1 = np.empty((B, N), np.float32)
    dist2 = np.empty((B, M), np.float32)
    for c in range(NCORES):
        b, h = divmod(c, 2)
        dist1[b, h * NLOC : (h + 1) * NLOC] = np.asarray(results[c]["d1"]).T.ravel()
        d2p = np.asarray(results[c]["d2"]).T.ravel()
        if h == 0:
            dist2[b] = d2p
        else:
            np.minimum(dist2[b], d2p, out=dist2[b])
    return dist1, dist2


def kernel(xyz1, xyz2):
    from concourse.bass_utils import run_bass_kernel_spmd

    nc = _get_nc()
    in_maps = _make_in_maps(xyz1, xyz2)
    res = run_bass_kernel_spmd(nc, in_maps, core_ids=list(range(NCORES)))
    return _unshard(res.results)
